# revision 80
# baseline (speedup 1.0000x reference)
"""Trainium2 Bass kernel for MllamaTextSdpaAttention (GQA + RoPE + causal SDPA).

Tensor-parallel over heads across 8 NeuronCores. Core c owns q-heads
[4c, 4c+4) and kv-head c; partial [T, DIM] outputs are summed on the host.
Scores are computed transposed (scT[k, q]) so exp(scT) feeds P@V directly.

All four projection GEMMs (Q/K/V/O) run as fp8-e4m3 DoubleRow matmuls
(256-deep contraction, 0.5 cycles/row) with an error-compensating hi/lo
split: x ~= hi + lo with both parts e4m3 (e4m3's exponent range makes the
split exact to ~2^-8 with no scale factors), and out = hi@hi + hi@lo +
lo@hi, dropping the negligible lo@lo term. 3 passes x 0.5 = 0.75x the bf16
cycle cost. Weights are quantized x32 (their std 0.02 sits at e4m3's
subnormal floor); the x32 is compensated in the rope tables (/32), the V
and O PSUM->SBUF copies (x1/32), with 1/sqrt(d) folded into the exp's
scale argument and the additive mask host-scaled by sqrt(d). hs and the
device-side attention tensors (qt/kt/v/exp tiles) stay bf16; the
attention-output ao is split hi/lo on DVE for the O-projection.

Scheduling (inherited from the bf16 tuning): softmax rowsums via DVE
tile-accumulation + gpsimd partition_all_reduce (no PE ones-matmuls, rowsum
normalize deferred one group); causal mask applied on the PE as an
accumulating identity.T @ maskT matmul; attention groups of chunk c
interleave with the projection chains of chunk c+1 via a time-budgeted
filler generator; weights land in [128, ...] SBUF-image layouts with
startup DMAs slab-sliced in consumption order; the last Q chain reads
hs_lo in its first pass so the single-buffered lo tile frees early for
the next chunk's DMA; paired output tiles share one DMA each to halve
HWDGE issue overhead in the O-projection phase.
"""

import numpy as np
import ml_dtypes

import concourse.bacc as bacc
import concourse.bass as bass
import concourse.mybir as mybir
from concourse import bass_isa
from concourse.tile import TileContext
from concourse import bass_utils

BF16 = mybir.dt.bfloat16
F32 = mybir.dt.float32
F8 = mybir.dt.float8e4
DR = mybir.MatmulPerfMode.DoubleRow

B, S, DIM = 2, 1024, 4096
T = B * S                     # 2048 tokens, batch-major
N_HEADS, N_KV = 32, 8
HD = 128
N_CORES = 8
HL = N_HEADS // N_CORES       # 4 local q-heads per core
KT = DIM // 128               # 32 feature tiles
KP = KT // 2                  # 16 DoubleRow k-tile pairs
CH = 512                      # chunk width (tokens)
NCHUNK = T // CH
QB = 512
TT = T // 128                 # 16 token tiles
SCALE = 1.0 / float(np.sqrt(HD))
WS = 32.0                     # weight quantization scale

_CACHE: dict = {}


def _build():
    nc = bacc.Bacc("TRN2", target_bir_lowering=False, debug=False,
                   enable_asserts=False)

    hs_hi = nc.dram_tensor("hs_hi", [DIM, T], F8, kind="ExternalInput")
    hs_lo = nc.dram_tensor("hs_lo", [DIM, T], F8, kind="ExternalInput")
    wq_hi = nc.dram_tensor("wq_hi", [128, HL, KT, HD], F8, kind="ExternalInput")
    wq_lo = nc.dram_tensor("wq_lo", [128, HL, KT, HD], F8, kind="ExternalInput")
    wk_hi = nc.dram_tensor("wk_hi", [128, KT, HD], F8, kind="ExternalInput")
    wk_lo = nc.dram_tensor("wk_lo", [128, KT, HD], F8, kind="ExternalInput")
    wv_hi = nc.dram_tensor("wv_hi", [128, KT, HD], F8, kind="ExternalInput")
    wv_lo = nc.dram_tensor("wv_lo", [128, KT, HD], F8, kind="ExternalInput")
    wo_hi = nc.dram_tensor("wo_hi", [128, HL, DIM], F8, kind="ExternalInput")
    wo_lo = nc.dram_tensor("wo_lo", [128, HL, DIM], F8, kind="ExternalInput")
    cos_d = nc.dram_tensor("cos_d", [HD, T], BF16, kind="ExternalInput")
    sin_d = nc.dram_tensor("sin_d", [HD, T], BF16, kind="ExternalInput")
    maskT = nc.dram_tensor("maskT", [128, 128], BF16, kind="ExternalInput")
    ident = nc.dram_tensor("ident", [128, 128], BF16, kind="ExternalInput")
    out = nc.dram_tensor("out", [T, DIM], BF16, kind="ExternalOutput")

    Exp = mybir.ActivationFunctionType.Exp

    with TileContext(nc) as tc:
        with tc.tile_pool(name="consts", bufs=1) as cpool, \
             tc.tile_pool(name="hs", bufs=2) as hpool, \
             tc.tile_pool(name="rope_tmp", bufs=1) as rpool, \
             tc.tile_pool(name="work_ps", bufs=5, space=bass.MemorySpace.PSUM) as wpool, \
             tc.tile_pool(name="ot_ps", bufs=3, space=bass.MemorySpace.PSUM) as otpool, \
             tc.tile_pool(name="et", bufs=4) as epool, \
             tc.tile_pool(name="ssum", bufs=2) as spool, \
             tc.tile_pool(name="rsbc", bufs=1) as rbpool, \
             tc.tile_pool(name="recip", bufs=1) as rcpool, \
             tc.tile_pool(name="out_sb", bufs=5) as xsbpool:

            wqh = [cpool.tile([128, KT, HD], F8, tag=f"wqh{m}", name=f"wqh{m}")
                   for m in range(HL)]
            wql = [cpool.tile([128, KT, HD], F8, tag=f"wql{m}", name=f"wql{m}")
                   for m in range(HL)]
            wkh_sb = cpool.tile([128, KT, HD], F8, tag="wkh")
            wkl_sb = cpool.tile([128, KT, HD], F8, tag="wkl")
            wvh_sb = cpool.tile([128, KT, HD], F8, tag="wvh")
            wvl_sb = cpool.tile([128, KT, HD], F8, tag="wvl")
            woh_sb = cpool.tile([128, HL, DIM], F8, tag="woh")
            wol_sb = cpool.tile([128, HL, DIM], F8, tag="wol")
            cos_sb = cpool.tile([128, T], BF16, tag="cos")
            sin_sb = cpool.tile([128, T], BF16, tag="sin")
            maskT_sb = cpool.tile([128, 128], BF16, tag="maskT")
            ident_sb = cpool.tile([128, 128], BF16, tag="ident")
            qt_rot = cpool.tile([128, HL, T], BF16, tag="qt")
            kt_rot = cpool.tile([128, T], BF16, tag="kt")
            v_sb = cpool.tile([128, TT, HD], BF16, tag="v")
            aoh = cpool.tile([128, HL, T], F8, tag="aoh")
            aol = cpool.tile([128, HL, T], F8, tag="aol")

            hsh_r = hs_hi.ap().rearrange("(kt p) t -> p kt t", p=128)
            hsl_r = hs_lo.ap().rearrange("(kt p) t -> p kt t", p=128)
            hs_tiles: dict = {}

            def issue_hs(c, part, slabs=None):
                """DMA one [128, KT, CH] fp8 hs chunk part ('h'/'l')."""
                t0 = c * CH
                src = hsh_r if part == 'h' else hsl_r
                tile = hpool.tile([128, KT, CH], F8, tag=f"hs{part}",
                                  bufs=(2 if part == 'h' else 1),
                                  name=f"hs{part}{c}")
                hs_tiles[(c, part)] = tile
                if slabs is None:
                    slabs = (16, 16)
                k0 = 0
                for w in slabs:
                    nc.sync.dma_start(tile[:, k0:k0 + w, :],
                                      src[:, k0:k0 + w, t0:t0 + CH])
                    k0 += w
                assert k0 == KT

            def rope(ps, out_ap, t0):
                """out = ps*cos + halfswap(ps)*sin (signs baked into sin)."""
                c_ap = cos_sb[:, t0:t0 + CH]
                s_ap = sin_sb[:, t0:t0 + CH]
                t2 = rpool.tile([128, CH], F32, tag="r2", name="t2")
                nc.vector.tensor_mul(t2[0:64, :], ps[64:128, :], s_ap[0:64, :])
                nc.vector.tensor_mul(t2[64:128, :], ps[0:64, :], s_ap[64:128, :])
                nc.vector.tensor_mul(out_ap, ps, c_ap)
                nc.vector.tensor_add(out_ap, out_ap, t2)

            def proj_gen(c, q_heads=tuple(range(HL))):
                """fp8 DoubleRow 3-pass projection chains for chunk c
                (K -> V -> Q); yields PE-ns after each matmul."""
                t0 = c * CH
                hh = hs_tiles[(c, 'h')]
                hl = hs_tiles[(c, 'l')]
                # K projection
                ps = wpool.tile([128, CH], F32, tag="work", name="ps_k")
                passes = [(wkh_sb, hh), (wkl_sb, hh), (wkh_sb, hl)]
                n = 3 * KP
                i = 0
                for wt, xt in passes:
                    for kp in range(KP):
                        nc.tensor.matmul(ps, wt[:, 2 * kp:2 * kp + 2, :],
                                         xt[:, 2 * kp:2 * kp + 2, :],
                                         start=(i == 0), stop=(i == n - 1),
                                         perf_mode=DR)
                        i += 1
                        yield CH * 0.2083
                rope(ps, kt_rot[:, t0:t0 + CH], t0)
                # V projection (hs stationary, wv moving)
                for vi in range(CH // 128):
                    tt = t0 // 128 + vi
                    ps = wpool.tile([128, HD], F32, tag="work", name="ps_v")
                    vpasses = [(hh, wvh_sb), (hh, wvl_sb), (hl, wvh_sb)]
                    i = 0
                    for xt, wt in vpasses:
                        for kp in range(KP):
                            nc.tensor.matmul(
                                ps, xt[:, 2 * kp:2 * kp + 2,
                                       vi * 128:(vi + 1) * 128],
                                wt[:, 2 * kp:2 * kp + 2, :],
                                start=(i == 0), stop=(i == n - 1),
                                perf_mode=DR)
                            i += 1
                            yield HD * 0.2083
                    nc.scalar.mul(v_sb[:, tt, :], ps, 1.0 / WS)
                # Q projections
                for m in q_heads:
                    ps = wpool.tile([128, CH], F32, tag="work", name="ps_q")
                    if m == HL - 1:
                        # last chain touches hs_lo first so its (single)
                        # buffer frees early for the next chunk's DMA
                        qpasses = [(wqh[m], hl), (wql[m], hh), (wqh[m], hh)]
                    else:
                        qpasses = [(wqh[m], hh), (wql[m], hh), (wqh[m], hl)]
                    i = 0
                    for wt, xt in qpasses:
                        for kp in range(KP):
                            nc.tensor.matmul(ps, wt[:, 2 * kp:2 * kp + 2, :],
                                             xt[:, 2 * kp:2 * kp + 2, :],
                                             start=(i == 0), stop=(i == n - 1),
                                             perf_mode=DR)
                            i += 1
                            yield CH * 0.2083
                    rope(ps, qt_rot[:, m, t0:t0 + CH], t0)

            def oproj_gen(tts, split_last=False):
                """fp8 DoubleRow 3-pass output projection; yields PE-ns."""
                last = (tts[-1], DIM // 512 - 1)
                for tt in tts:
                    for ni in range(DIM // 512):
                        ps = wpool.tile([128, 512], F32, tag="work", name="ps_o")
                        opasses = [(aoh, woh_sb), (aoh, wol_sb), (aol, woh_sb)]
                        i = 0
                        for at, wt in opasses:
                            for khp in range(HL // 2):
                                nc.tensor.matmul(
                                    ps,
                                    at[:, 2 * khp:2 * khp + 2,
                                       tt * 128:(tt + 1) * 128],
                                    wt[:, 2 * khp:2 * khp + 2,
                                       ni * 512:(ni + 1) * 512],
                                    start=(i == 0), stop=(i == 5),
                                    perf_mode=DR)
                                i += 1
                                yield 512 * 0.2083
                        if split_last and (tt, ni + 1) == last:
                            osb = xsbpool.tile([128, 1024], BF16, tag="osb",
                                               name="osb")
                            nc.scalar.mul(osb[:, 0:512], ps, 1.0 / WS)
                            nc.sync.dma_start(
                                out.ap()[tt * 128:(tt + 1) * 128,
                                         ni * 512:(ni + 1) * 512],
                                osb[:, 0:512])
                        elif split_last and (tt, ni) == last:
                            nc.vector.tensor_scalar_mul(osb[:, 512:], ps,
                                                        1.0 / WS)
                            nc.sync.dma_start(
                                out.ap()[tt * 128:(tt + 1) * 128,
                                         ni * 512:(ni + 1) * 512],
                                osb[:, 512:])
                        elif ni % 2 == 0:
                            osb = xsbpool.tile([128, 1024], BF16, tag="osb",
                                               name="osb")
                            nc.scalar.mul(osb[:, 0:512], ps, 1.0 / WS)
                        else:
                            nc.vector.tensor_scalar_mul(osb[:, 512:], ps,
                                                        1.0 / WS)
                            nc.sync.dma_start(
                                out.ap()[tt * 128:(tt + 1) * 128,
                                         (ni - 1) * 512:(ni + 1) * 512], osb)

            def mk_filler(gen):
                state = {'bank': 0.0, 'done': False}

                def filler(ns):
                    state['bank'] -= ns
                    while state['bank'] < 0 and not state['done']:
                        got = next(gen, None)
                        if got is None:
                            state['done'] = True
                            return
                        state['bank'] += got
                return filler

            def drain(gen):
                for _ in gen:
                    pass

            pending = []

            def flush_pending():
                while pending:
                    rb, ot, h, q0 = pending.pop(0)
                    rc = rcpool.tile([128, QB], F32, tag="rc", name="rc")
                    nc.vector.reciprocal(rc, rb)
                    # rb is dead after the reciprocal: reuse it for the
                    # normalized product (DVE is in-order, WAR is safe)
                    nc.vector.tensor_mul(rb, ot, rc)
                    nc.vector.tensor_copy(aoh[:, h, q0:q0 + QB], rb)
                    nc.vector.tensor_sub(aol[:, h, q0:q0 + QB], rb,
                                         aoh[:, h, q0:q0 + QB])

            def emit_group(b, h, qb, filler):
                """Attention for one q-head block: transposed scores, bf16."""
                q0 = b * S + qb * QB
                n_kt = (qb + 1) * (QB // 128)
                ot = otpool.tile([128, QB], F32, tag="ot", name="ot")
                sacc = spool.tile([128, QB], F32, tag="S", name="sacc")
                ets = {}
                W = 3

                def emit_sc(kt):
                    c0 = max(0, kt - qb * (QB // 128)) * 128
                    w = QB - c0
                    sc = wpool.tile([128, QB], F32, tag="work", name="sc")
                    jd = kt - qb * (QB // 128)
                    diag = 0 <= jd < QB // 128
                    nc.tensor.matmul(
                        sc[:, c0:],
                        kt_rot[:, b * S + kt * 128:b * S + (kt + 1) * 128],
                        qt_rot[:, h, q0 + c0:q0 + QB],
                        start=True, stop=not diag, skip_group_check=diag)
                    pe = w * 0.4166
                    if diag:
                        nc.tensor.matmul(sc[:, jd * 128:(jd + 1) * 128],
                                         ident_sb, maskT_sb,
                                         start=False, stop=True,
                                         skip_group_check=True)
                        pe += 128 * 0.4166
                    et = epool.tile([128, QB], BF16, tag="et", name="et")
                    nc.scalar.activation(et[:, c0:], sc[:, c0:], Exp,
                                         bias=0.0, scale=SCALE)
                    ets[kt] = (et, c0)
                    return (w * 0.8333 + 230) - pe

                deficit = 0.0
                for w in range(min(W, n_kt)):
                    deficit += emit_sc(w)
                filler(max(0.0, deficit) + 4000.0)
                for kt in range(n_kt):
                    d = 0.0
                    if kt + W < n_kt:
                        d += emit_sc(kt + W)
                    et, c0 = ets.pop(kt)
                    nc.tensor.matmul(ot[:, c0:], v_sb[:, b * (S // 128) + kt, :],
                                     et[:, c0:], start=(kt == 0),
                                     stop=(kt == n_kt - 1))
                    d -= (QB - c0) * 0.4166
                    if kt == 0:
                        nc.vector.tensor_copy(sacc, et)
                        flush_pending()
                    else:
                        nc.vector.tensor_add(sacc[:, c0:], sacc[:, c0:],
                                             et[:, c0:])
                    filler(max(0.0, d) + 400.0)
                rb = rbpool.tile([128, QB], F32, tag="rb", name="rb")
                nc.gpsimd.partition_all_reduce(rb, sacc, 128,
                                               bass_isa.ReduceOp.add)
                pending.append((rb, ot, h, q0))

            # ---- startup: slab-sliced DMAs in consumption order ----
            wqh_r, wql_r = wq_hi.ap(), wq_lo.ap()
            nc.sync.dma_start(wkh_sb, wk_hi.ap())
            issue_hs(0, 'h', slabs=(8, 8, 16))
            nc.sync.dma_start(wkl_sb, wk_lo.ap())
            nc.sync.dma_start(wvh_sb, wv_hi.ap())
            nc.sync.dma_start(wvl_sb, wv_lo.ap())
            issue_hs(0, 'l', slabs=(16, 16))
            nc.sync.dma_start(cos_sb[:, 0:CH], cos_d.ap()[:, 0:CH])
            nc.sync.dma_start(sin_sb[:, 0:CH], sin_d.ap()[:, 0:CH])
            for m in range(HL):
                nc.sync.dma_start(wqh[m], wqh_r[:, m, :, :])
                nc.sync.dma_start(wql[m], wql_r[:, m, :, :])
            nc.sync.dma_start(maskT_sb, maskT.ap())
            nc.sync.dma_start(ident_sb, ident.ap())
            issue_hs(1, 'h')
            issue_hs(1, 'l')
            nc.sync.dma_start(cos_sb[:, CH:], cos_d.ap()[:, CH:])
            nc.sync.dma_start(sin_sb[:, CH:], sin_d.ap()[:, CH:])

            # ---- phase 0: chunk-0 projections (DMA-paced) ----
            drain(proj_gen(0))

            # ---- phases 1..3: groups of chunk c-1 + projections of chunk c
            for c in range(1, NCHUNK):
                if c + 1 < NCHUNK:
                    issue_hs(c + 1, 'h')
                    issue_hs(c + 1, 'l')
                if c == NCHUNK - 1:
                    nc.sync.dma_start(woh_sb, wo_hi.ap())
                    nc.sync.dma_start(wol_sb, wo_lo.ap())
                g = proj_gen(c)
                fill = mk_filler(g)
                pb, pqb = (c - 1) // 2, (c - 1) % 2
                for h in range(HL):
                    emit_group(pb, h, pqb, fill)
                drain(g)

            # ---- phase 4: last chunk's groups + early O-proj tiles ----
            og = oproj_gen(list(range(TT)), split_last=True)
            fill = mk_filler(og)
            pb, pqb = (NCHUNK - 1) // 2, (NCHUNK - 1) % 2
            for h in range(HL):
                emit_group(pb, h, pqb, fill)
            flush_pending()
            # ---- phase 5: rest of the output projection ----
            drain(og)
    nc.compile()
    return nc


def _get_nc():
    if "nc" not in _CACHE:
        _CACHE["nc"] = _build()
    return _CACHE["nc"]


def _split8(x):
    f8 = ml_dtypes.float8_e4m3
    hi = x.astype(f8)
    lo = (x - hi.astype(np.float32)).astype(f8)
    return np.ascontiguousarray(hi), np.ascontiguousarray(lo)


def _prep_inputs(inputs) -> list[dict]:
    bf16 = ml_dtypes.bfloat16
    hs = np.asarray(inputs["hidden_states"], dtype=np.float32).reshape(T, DIM)
    hsT = np.ascontiguousarray(hs.T)
    hsT_hi, hsT_lo = _split8(hsT)

    fc = np.asarray(inputs["freqs_cos"], dtype=np.float32).reshape(T, HD // 2).T
    fs = np.asarray(inputs["freqs_sin"], dtype=np.float32).reshape(T, HD // 2).T
    cos2 = np.concatenate([fc, fc], axis=0) / WS       # [128, T]
    sin2 = np.concatenate([-fs, fs], axis=0) / WS      # signed half-rotation
    cos_v = np.ascontiguousarray(cos2).astype(bf16)
    sin_v = np.ascontiguousarray(sin2).astype(bf16)

    # mask is added to true-scale scores pre-exp; exp applies x SCALE
    maskT = np.ascontiguousarray(
        np.asarray(inputs["attention_mask"],
                   dtype=np.float32)[0, 0, :128, :128].T / SCALE).astype(bf16)
    ident = np.eye(128, dtype=np.float32).astype(bf16)

    perm = np.concatenate([np.arange(0, HD, 2), np.arange(1, HD, 2)])
    Wq = np.asarray(inputs["Wq"], dtype=np.float32) * WS
    Wk = np.asarray(inputs["Wk"], dtype=np.float32) * WS
    Wv = np.asarray(inputs["Wv"], dtype=np.float32) * WS
    Wo = np.asarray(inputs["Wo"], dtype=np.float32) * WS

    in_maps = []
    for c in range(N_CORES):
        wq_c = np.concatenate(
            [Wq[:, (c * HL + h) * HD:(c * HL + h + 1) * HD][:, perm]
             for h in range(HL)], axis=1)               # [DIM, HL*HD]
        wk_c = Wk[:, c * HD:(c + 1) * HD][:, perm]      # [DIM, HD]
        wv_c = Wv[:, c * HD:(c + 1) * HD]
        wo_c = Wo[c * HL * HD:(c + 1) * HL * HD, :]     # [HL*HD, DIM]
        # [128, ...] SBUF-image layouts
        wq_img = wq_c.reshape(KT, 128, HL, HD).transpose(1, 2, 0, 3)
        wk_img = wk_c.reshape(KT, 128, HD).transpose(1, 0, 2)
        wv_img = wv_c.reshape(KT, 128, HD).transpose(1, 0, 2)
        wo_img = wo_c.reshape(HL, 128, DIM).transpose(1, 0, 2)
        wq_hi, wq_lo = _split8(wq_img)
        wk_hi, wk_lo = _split8(wk_img)
        wv_hi, wv_lo = _split8(wv_img)
        wo_hi, wo_lo = _split8(wo_img)
        in_maps.append({
            "hs_hi": hsT_hi, "hs_lo": hsT_lo,
            "wq_hi": wq_hi, "wq_lo": wq_lo,
            "wk_hi": wk_hi, "wk_lo": wk_lo,
            "wv_hi": wv_hi, "wv_lo": wv_lo,
            "wo_hi": wo_hi, "wo_lo": wo_lo,
            "cos_d": cos_v, "sin_d": sin_v,
            "maskT": maskT, "ident": ident,
        })
    return in_maps


def kernel(**inputs) -> np.ndarray:
    nc = _get_nc()
    in_maps = _prep_inputs(inputs)
    res = bass_utils.run_bass_kernel_spmd(nc, in_maps,
                                          core_ids=list(range(N_CORES)))
    acc = np.zeros((T, DIM), dtype=np.float32)
    for c in range(N_CORES):
        acc += np.asarray(res.results[c]["out"], dtype=np.float32)
    return acc.reshape(B, S, DIM)
